# revision 22
# baseline (speedup 1.0000x reference)
"""Trainium2 Bass kernel v3 for nn_AttentionLayer (sparse graph attention + BN).

Strategy (8 cores, SPMD):
  - Host assigns nodes to 784 windows of 128 slots (serpentine deal by degree,
    plus a repair pass) so every window owns <= tpw*128 incident edges; 98
    windows per core. Output rows are produced in this "virtual" order and
    un-permuted on the host at the end.
  - Blob is stored partition-major ([W, 128, tpw*896]) so each window's DMA is
    one fully-contiguous 1.1MB transfer (8960B per partition line).
  - Device pipeline per 2-tile group: 6 bf16 matmuls/tile (KV, Q, EH
    projections + segment-sum), one scalar PSUM->SBUF copy for the EH|Q bank,
    score pipeline fused over 2 tiles on vector/scalar/gpsimd.
  - BatchNorm: per-core partial sums accumulated on the PE; host reduces
    across cores between the two launches; a second kernel applies the affine
    transform in transposed layout (per-partition scale/bias on the scalar
    engine), writing fp16.
"""

import math
import numpy as np
import ml_dtypes

import concourse.bass as bass
import concourse.tile as tile
from concourse import mybir
from concourse.bass_utils import run_bass_kernel_spmd

F32 = mybir.dt.float32
F16 = mybir.dt.float16
BF16 = mybir.dt.bfloat16

CORES = 8
N = 100000
E = 500000
DIM = 256
H = 8
DH = DIM // H
WPC = 98                  # windows per core
W = CORES * WPC           # 784 windows of 128 slots
NV = W * 128              # 100352 virtual node slots
NPC = WPC * 128           # 12544 node slots per core
EPS_Z = 1e-6
EPS_BN = 1e-5


# ----------------------------------------------------------------- host prep

def _balance_windows(deg):
    """Assign nodes to W windows of <=128 slots, balancing edge (degree) sums.
    Serpentine deal over degree-sorted nodes, then a swap repair pass.
    Returns (win_of_node, pos_of_node, max_sum)."""
    order = np.argsort(-deg, kind="stable")
    win_of = np.empty(N, dtype=np.int64)
    pos_of = np.empty(N, dtype=np.int64)
    rows = math.ceil(N / W)
    for r in range(rows):
        chunk = order[r * W:(r + 1) * W]
        cols = np.arange(chunk.shape[0])
        if r % 2 == 1:
            cols = W - 1 - cols
        win_of[chunk] = cols
        pos_of[chunk] = r
    sums = np.bincount(win_of, weights=deg, minlength=W).astype(np.int64)
    counts = np.bincount(win_of, minlength=W)

    cap = 128 * max(1, math.ceil(sums.mean() / 128))
    if sums.max() > cap:
        members = [list(np.where(win_of == w)[0]) for w in range(W)]
        for _ in range(5000):
            hi = int(np.argmax(sums))
            if sums[hi] <= cap:
                break
            need = sums[hi] - cap
            cands = sorted(members[hi], key=lambda n: deg[n])
            cand = next((n for n in cands if deg[n] >= need), cands[-1])
            d = deg[cand]
            blocked = (counts >= 128) | (sums + d > cap)
            blocked[hi] = True
            if blocked.all():
                break
            lo = int(np.argmin(np.where(blocked, np.iinfo(np.int64).max, sums)))
            members[hi].remove(cand)
            members[lo].append(cand)
            win_of[cand] = lo
            sums[hi] -= d
            sums[lo] += d
            counts[hi] -= 1
            counts[lo] += 1
    pos_of = np.zeros(N, dtype=np.int64)
    order2 = np.argsort(win_of, kind="stable")
    start = 0
    for w, c in enumerate(np.bincount(win_of, minlength=W)):
        pos_of[order2[start:start + c]] = np.arange(c)
        start += c
    return win_of, pos_of, int(sums.max())


def _prep(x, edge_attr, edge_index):
    src = np.asarray(edge_index[0], dtype=np.int64)
    dst = np.asarray(edge_index[1], dtype=np.int64)
    x = np.asarray(x, dtype=np.float32)
    edge_attr = np.asarray(edge_attr, dtype=np.float32)

    deg = np.bincount(dst, minlength=N).astype(np.int64)
    win_of, pos_of, max_sum = _balance_windows(deg)
    tpw = max(1, math.ceil(max_sum / 128))
    T = WPC * tpw            # tiles per core
    vid = win_of * 128 + pos_of

    # edge -> (core, tile, lane)
    wd = win_of[dst]
    order_e = np.argsort(wd, kind="stable")
    wds = wd[order_e]
    cnt = np.bincount(wd, minlength=W)
    starts = np.zeros(W, dtype=np.int64)
    starts[1:] = np.cumsum(cnt)[:-1]
    pos_in_w = np.arange(E, dtype=np.int64) - starts[wds]
    tile_in_w = pos_in_w >> 7
    lane = pos_in_w & 127
    core_e = wds // WPC
    flat_t = core_e * T + (wds % WPC) * tpw + tile_in_w  # [E] in sorted order

    TT_all = CORES * T
    srcs = np.zeros((TT_all, 128), dtype=np.int64)
    slots = np.zeros((TT_all, 128), dtype=np.int64)
    valid = np.zeros((TT_all, 128), dtype=bool)
    eidx = np.zeros((TT_all, 128), dtype=np.int64)
    srcs[flat_t, lane] = src[order_e]
    slots[flat_t, lane] = pos_of[dst[order_e]]
    eidx[flat_t, lane] = order_e
    valid[flat_t, lane] = True

    xbf = x.astype(ml_dtypes.bfloat16)
    eabf = edge_attr.astype(ml_dtypes.bfloat16)
    xdst_nodes = np.zeros(NV, dtype=np.int64)  # virtual slot -> node (0 pad ok)
    node_mask = np.zeros(NV, dtype=bool)
    xdst_nodes[vid] = np.arange(N)
    node_mask[vid] = True

    per_core = []
    for c in range(CORES):
        sl = slice(c * T, (c + 1) * T)
        sc, vv, ei = srcs[sl], valid[sl], eidx[sl]
        st = slots[sl]
        blob = np.zeros((T, 128, 896), dtype=ml_dtypes.bfloat16)
        xs = xbf[sc]                       # [T,128,256]
        blob[:, :, 0:256] = np.concatenate(
            (xs[:, :, 0:128].transpose(0, 2, 1),
             xs[:, :, 128:256].transpose(0, 2, 1)), axis=2)
        wglob = (np.arange(T) // tpw) + c * WPC
        vslot = wglob[:, None] * 128 + st
        dsts_c = xdst_nodes[vslot]
        xd = xbf[dsts_c]
        blob[:, :, 256:512] = np.concatenate(
            (xd[:, :, 0:128].transpose(0, 2, 1),
             xd[:, :, 128:256].transpose(0, 2, 1)), axis=2)
        ea = eabf[ei]
        blob[:, :, 512:768] = np.concatenate(
            (ea[:, :, 0:128].transpose(0, 2, 1),
             ea[:, :, 128:256].transpose(0, 2, 1)), axis=2)
        selb = np.zeros((T, 128, 128), dtype=ml_dtypes.bfloat16)
        tI, lI = np.nonzero(vv)
        selb[tI, lI, st[tI, lI]] = 1.0
        blob[:, :, 768:896] = selb

        # partition-major layout: [WPC, 128, tpw*896], contiguous per partition
        blob_pm = np.ascontiguousarray(
            blob.reshape(WPC, tpw, 128, 896).transpose(0, 2, 1, 3)
        ).reshape(WPC, 128, tpw * 896)

        vsl = slice(c * NPC, (c + 1) * NPC)
        xwin = np.zeros((NPC, DIM), dtype=ml_dtypes.bfloat16)
        m = node_mask[vsl]
        xwin[m] = xbf[xdst_nodes[vsl][m]]
        per_core.append(dict(blob=blob_pm, xwin=xwin))
    return dict(per_core=per_core, tpw=tpw, vid=vid)


# ------------------------------------------------------------- phase1 kernel

def _groups(tpw):
    g, t = [], 0
    while t < tpw:
        n = min(2, tpw - t)
        g.append((t, n))
        t += n
    return g


def _build_phase1(tpw, split_waits=True):
    from contextlib import ExitStack
    nc = bass.Bass()
    blob_d = nc.declare_dram_parameter("blob", [WPC, 128, tpw * 896], BF16,
                                       isOutput=False)
    xwin_d = nc.declare_dram_parameter("xwin", [NPC, DIM], BF16, isOutput=False)
    wk_d = nc.declare_dram_parameter("wk", [128, 2, DIM], BF16, isOutput=False)
    wv_d = nc.declare_dram_parameter("wv", [128, 2, DIM], BF16, isOutput=False)
    wq_d = nc.declare_dram_parameter("wq", [128, 2, DIM], BF16, isOutput=False)
    we_d = nc.declare_dram_parameter("we", [128, 2, DIM], BF16, isOutput=False)
    hpre_d = nc.declare_dram_parameter("hpre", [NPC, DIM], BF16, isOutput=True)

    mul = mybir.AluOpType.mult
    add = mybir.AluOpType.add
    GRP = _groups(tpw)
    NGW = len(GRP)           # groups per window
    NG = WPC * NGW           # total groups

    with tile.TileContext(nc) as tc, ExitStack() as ctx:
        const = ctx.enter_context(tc.tile_pool(name="const", bufs=1))
        wk_sb = const.tile([128, 2, DIM], BF16, tag="wk")
        nc.sync.dma_start(wk_sb[:], wk_d[:])
        wv_sb = const.tile([128, 2, DIM], BF16, tag="wv")
        nc.sync.dma_start(wv_sb[:], wv_d[:])
        wq_sb = const.tile([128, 2, DIM], BF16, tag="wq")
        nc.sync.dma_start(wq_sb[:], wq_d[:])
        we_sb = const.tile([128, 2, DIM], BF16, tag="we")
        nc.sync.dma_start(we_sb[:], we_d[:])

        # PSUM: K (1 bank)x2 + V jit (1 bank)x1 + eq (2 banks)x2 + wvz = 8.
        # eq double-buffering breaks the slot-pacing cycle eh/q-matmuls ->
        # scalar copy -> next eh/q-matmuls on a single bank pair; the jit V
        # tile is produced and consumed within one slot so one bank suffices.
        pk = ctx.enter_context(tc.tile_pool(name="pk", bufs=2, space="PSUM"))
        pv = ctx.enter_context(tc.tile_pool(name="pv", bufs=1, space="PSUM"))
        peq = ctx.enter_context(tc.tile_pool(name="peq", bufs=2, space="PSUM"))
        pwvz = ctx.enter_context(tc.tile_pool(name="pwvz", bufs=1, space="PSUM"))

        p_blob = ctx.enter_context(tc.tile_pool(name="p_blob", bufs=3))
        p_eq = ctx.enter_context(tc.tile_pool(name="p_eq", bufs=4))
        p_m = ctx.enter_context(tc.tile_pool(name="p_m", bufs=5))
        p_small = ctx.enter_context(tc.tile_pool(name="p_small", bufs=6))
        p_msgz = ctx.enter_context(tc.tile_pool(name="p_msgz", bufs=4))
        p_h = ctx.enter_context(tc.tile_pool(name="p_h", bufs=3))
        p_xw = ctx.enter_context(tc.tile_pool(name="p_xw", bufs=3))

        def mm(out, lhsT, rhs, start, stop, **kw):
            nc.tensor.matmul(out, lhsT, rhs, start=start, stop=stop, **kw)

        state = {}

        def gidx(g):
            w, j = divmod(g, NGW)
            t0, n = GRP[j]
            return w, j, t0, n

        # stage A+B: window DMA + projections for the group's tiles
        def stage_B(g):
            w, j, t0, n = gidx(g)
            if j == 0:
                bw = p_blob.tile([128, tpw, 896], BF16, tag="blob")
                nc.sync.dma_start(
                    bw[:].rearrange("p t f -> p (t f)"), blob_d[w])
                xw = p_xw.tile([128, DIM], BF16, tag="xw")
                nc.sync.dma_start(xw[:], xwin_d[w * 128:(w + 1) * 128, :])
                state[("bw", w)] = bw
                state[("xw", w)] = xw
            bw = state[("bw", w)]
            k2 = pk.tile([128, 2, DIM], F32, tag="k2")
            eq2 = peq.tile([128, 2, 2 * DIM], F32, tag="eq2")
            for i in range(n):
                b = bw[:, t0 + i, :]
                mm(k2[:, i, :], b[:, 0:128], wk_sb[:, 0, :], True, False)
                mm(k2[:, i, :], b[:, 128:256], wk_sb[:, 1, :], False, True)
            for i in range(n):
                b = bw[:, t0 + i, :]
                mm(eq2[:, i, 0:DIM], b[:, 512:640], we_sb[:, 0, :], True, False)
                mm(eq2[:, i, 0:DIM], b[:, 640:768], we_sb[:, 1, :], False, True)
            for i in range(n):
                b = bw[:, t0 + i, :]
                mm(eq2[:, i, DIM:2 * DIM], b[:, 256:384], wq_sb[:, 0, :], True, False)
                mm(eq2[:, i, DIM:2 * DIM], b[:, 384:512], wq_sb[:, 1, :], False, True)
            state[("k2", g)] = k2
            state[("eq2", g)] = eq2

        # stage C: scalar copies EH|Q bank to SBUF
        def stage_C(g):
            w, j, t0, n = gidx(g)
            eq2 = state.pop(("eq2", g))
            eq_sb = p_eq.tile([128, 2, 2 * DIM], BF16, tag="eqsb")
            nc.scalar.copy(eq_sb[:, 0:n, :], eq2[:, 0:n, :])
            state[("eqsb", g)] = eq_sb

        # stage D: vector computes m1 = K * EH
        def stage_D(g):
            w, j, t0, n = gidx(g)
            k2 = state.pop(("k2", g))
            eq_sb = state[("eqsb", g)]
            m1 = p_m.tile([128, 2, DIM], BF16, tag="m1")
            nc.vector.tensor_tensor(out=m1[:, 0:n, :], in0=k2[:, 0:n, :],
                                    in1=eq_sb[:, 0:n, 0:DIM], op=mul)
            state[("m1", g)] = m1

        # stage E..H: gpsimd s2, vector reduce, gpsimd clip, scalar exp
        def stage_EH(g):
            w, j, t0, n = gidx(g)
            eq_sb = state.pop(("eqsb", g))
            m1 = state.pop(("m1", g))
            s2 = p_m.tile([128, 2, DIM], BF16, tag="s2")
            nc.gpsimd.tensor_tensor(out=s2[:, 0:n, :], in0=m1[:, 0:n, :],
                                    in1=eq_sb[:, 0:n, DIM:2 * DIM], op=mul)
            hs = p_small.tile([128, 2, H], F32, tag="hs")
            nc.vector.tensor_reduce(
                out=hs[:, 0:n, :, None],
                in_=s2[:, 0:n, :].rearrange("p n (h d) -> p n h d", d=DH),
                op=add, axis=mybir.AxisListType.X)
            hc = p_small.tile([128, 2, H], F32, tag="hc")
            nc.gpsimd.tensor_scalar(out=hc[:, 0:n, :], in0=hs[:, 0:n, :],
                                    scalar1=5.0, scalar2=-5.0,
                                    op0=mybir.AluOpType.min,
                                    op1=mybir.AluOpType.max)
            msgz = p_msgz.tile([128, 2, DIM + H], BF16, tag="msgz")
            nc.scalar.activation(msgz[:, 0:n, DIM:DIM + H], hc[:, 0:n, :],
                                 mybir.ActivationFunctionType.Exp)
            state[("msgz", g)] = msgz

        # stage I: PE (re)projects V just-in-time, vector applies scores
        def stage_I(g):
            w, j, t0, n = gidx(g)
            bw = state[("bw", w)]
            msgz = state.pop(("msgz", g))
            v2 = pv.tile([128, 2, DIM], F32, tag="v2")
            for i in range(n):
                b = bw[:, t0 + i, :]
                mm(v2[:, i, :], b[:, 0:128], wv_sb[:, 0, :], True, False)
                mm(v2[:, i, :], b[:, 128:256], wv_sb[:, 1, :], False, True)
            nc.vector.tensor_tensor(
                out=msgz[:, 0:n, 0:DIM].rearrange("p n (h d) -> p n h d", d=DH),
                in0=v2[:, 0:n, :].rearrange("p n (h d) -> p n h d", d=DH),
                in1=msgz[:, 0:n, DIM:DIM + H, None].to_broadcast([128, n, H, DH]),
                op=mul)
            state[("msgzf", g)] = msgz

        # stage J: segment-sum matmuls, one slot later so they never sit at
        # the PE queue head waiting for the msg-mult.
        def stage_J(g):
            w, j, t0, n = gidx(g)
            msgz = state.pop(("msgzf", g))
            if j == 0:
                state[("wvz", w)] = pwvz.tile([128, DIM + H], F32, tag="wvz",
                                              name="wvz")
            wvz = state[("wvz", w)]
            bw = state[("bw", w)]
            for i in range(n):
                t = t0 + i
                mm(wvz[:], bw[:, t, 768:896], msgz[:, i, :],
                   t == 0, t == tpw - 1)
            if j == NGW - 1:
                finalize(w)

        def finalize(w):
            wvz = state.pop(("wvz", w))
            state.pop(("bw", w))
            xw = state.pop(("xw", w))
            zr = p_small.tile([128, H], F32, tag="zr")
            nc.vector.tensor_scalar(out=zr[:], in0=wvz[:, DIM:DIM + H],
                                    scalar1=EPS_Z, scalar2=None, op0=add)
            nc.vector.reciprocal(zr[:], zr[:])
            h = p_h.tile([128, DIM], BF16, tag="h")
            nc.vector.tensor_tensor(
                out=h[:].rearrange("p (h d) -> p h d", d=DH),
                in0=wvz[:, 0:DIM].rearrange("p (h d) -> p h d", d=DH),
                in1=zr[:, :, None].to_broadcast([128, H, DH]), op=mul)
            nc.gpsimd.tensor_tensor(out=h[:], in0=h[:], in1=xw[:], op=add)
            nc.sync.dma_start(hpre_d[w * 128:(w + 1) * 128, :], h[:])

        # software pipeline over groups: B@0, C/D@1, EH/I@2, J@3. Issue order
        # per slot keeps each engine's FIFO free of head-of-line blocking:
        # the scalar copy C(i-1) is ready immediately and precedes exp; the
        # vector queue runs reduce(i-2), msgz(i-2) before m1(i-1) (which
        # waits on C); the PE runs V-mms(i-2), then the new projections, and
        # only then seg-sums(i-3), whose msg inputs are long since done.
        for i in range(NG + 3):
            if 1 <= i <= NG:
                stage_C(i - 1)
            if 2 <= i <= NG + 1:
                stage_EH(i - 2)
            if 2 <= i <= NG + 1:
                stage_I(i - 2)
            if 1 <= i <= NG:
                stage_D(i - 1)
            if i < NG:
                stage_B(i)
            if i >= 3:
                stage_J(i - 3)

    return _split_excess_waits(nc) if split_waits else nc


def _split_excess_waits(nc, max_waits=1):
    """Most HW-decoded opcodes carry only ~1 sync wait; move the excess onto
    preceding same-engine NoOps, which use the sequencer wait table."""
    k = 0
    skip = {"InstNoOp"}
    for f in nc.m.functions:
        for b in f.blocks:
            new = []
            for inst in b.instructions:
                si = inst.sync_info
                if (type(inst).__name__ not in skip and si is not None
                        and si.on_wait and len(si.on_wait) > max_waits):
                    extra = si.on_wait[:-max_waits]
                    for wt in extra:
                        nop = mybir.InstNoOp(name=f"I-wsplit{k}", ins=[], outs=[])
                        k += 1
                        nop.engine = inst.engine
                        nop.bass_nofuse = True
                        nop.sync_info = mybir.SyncInfo(on_wait=[wt], on_update=[])
                        new.append(nop)
                    inst.sync_info = mybir.SyncInfo(
                        on_wait=si.on_wait[-max_waits:], on_update=si.on_update)
                new.append(inst)
            b.instructions = new
    return nc


# ------------------------------------------------------------- phase2 kernel
# BN affine apply in transposed layout: hT[p, c, node] = h[node, c*128+p].
# scale/shift become per-partition vectors -> one scalar-engine ACTIVATE per
# (chunk c, node-slice), out = Identity(scale*in + bias) written as fp16.

P2_CHUNKS = 8


def _build_phase2():
    from contextlib import ExitStack
    nc = bass.Bass()
    hT_d = nc.declare_dram_parameter("hT", [128, 2, NPC], BF16, isOutput=False)
    sc_d = nc.declare_dram_parameter("scaleT", [128, 2], F32, isOutput=False)
    sh_d = nc.declare_dram_parameter("shiftT", [128, 2], F32, isOutput=False)
    out_d = nc.declare_dram_parameter("outT", [128, 2, NPC], F16, isOutput=True)
    CH = NPC // P2_CHUNKS
    assert CH * P2_CHUNKS == NPC
    with tile.TileContext(nc) as tc, ExitStack() as ctx:
        const = ctx.enter_context(tc.tile_pool(name="const", bufs=1))
        sc = const.tile([128, 2], F32, tag="sc")
        nc.sync.dma_start(sc[:], sc_d[:])
        sh = const.tile([128, 2], F32, tag="sh")
        nc.sync.dma_start(sh[:], sh_d[:])
        pool = ctx.enter_context(tc.tile_pool(name="ht", bufs=4))
        for b in range(P2_CHUNKS):
            ht = pool.tile([128, 2, CH], BF16, tag="ht")
            nc.sync.dma_start(ht[:], hT_d[:, :, b * CH:(b + 1) * CH])
            hf = pool.tile([128, 2, CH], F16, tag="hf")
            for c in range(2):
                nc.scalar.activation(hf[:, c, :], ht[:, c, :],
                                     mybir.ActivationFunctionType.Identity,
                                     bias=sh[:, c:c + 1], scale=sc[:, c:c + 1])
            nc.gpsimd.dma_start(out_d[:, :, b * CH:(b + 1) * CH], hf[:])
    return _split_excess_waits(nc)


# ------------------------------------------------------------------- runner

def _install_ntff_hook():
    import sys, types
    if "antenv.axon_hooks" in sys.modules:
        return True
    try:
        import antenv
        from trn_agent_boot.trn_boot import _ntff_profile_via_ctypes
        mod = types.ModuleType("antenv.axon_hooks")
        mod._hook = _ntff_profile_via_ctypes("/opt/axon/libaxon_pjrt.so")
        mod.set_axon_ntff_profile_hook = lambda h: setattr(mod, "_hook", h)
        mod.get_axon_ntff_profile_hook = lambda: mod._hook
        sys.modules["antenv.axon_hooks"] = mod
        antenv.axon_hooks = mod
        return mod._hook is not None
    except Exception:
        return False


_CACHE = {}


def _get_phase1(tpw):
    key = ("p1", tpw)
    if key not in _CACHE:
        _CACHE[key] = _build_phase1(tpw)
    return _CACHE[key]


def _get_phase2():
    key = ("p2",)
    if key not in _CACHE:
        _CACHE[key] = _build_phase2()
    return _CACHE[key]


def run_pipeline(x, edge_attr, WQ, WK, WE, WV, gamma, beta, edge_index,
                 timed=False):
    prep = _prep(x, edge_attr, edge_index)
    tpw = prep["tpw"]
    scale_inv = np.float32(1.0 / math.sqrt(DH))

    def cast_pm(a, f):
        # [DIM, f] weights -> [128, 2, f] partition-major bf16
        return np.ascontiguousarray(
            np.asarray(a, np.float32).reshape(2, 128, f).transpose(1, 0, 2)
        ).astype(ml_dtypes.bfloat16)

    wq = cast_pm(np.asarray(WQ, np.float32) * scale_inv, DIM)
    we = cast_pm(WE, DIM)
    wk = cast_pm(WK, DIM)
    wv = cast_pm(WV, DIM)

    nc1 = _get_phase1(tpw)
    in_maps = []
    for c in range(CORES):
        pc = prep["per_core"][c]
        in_maps.append(dict(blob=pc["blob"], xwin=pc["xwin"],
                            wk=wk, wv=wv, wq=wq, we=we))

    trace = timed and _install_ntff_hook()
    r1 = run_bass_kernel_spmd(nc1, in_maps, list(range(CORES)), trace=trace)
    results1 = r1.results
    t1 = r1.exec_time_ns
    # BN batch stats: reduce the (host-visible) hpre partial sums across
    # cores; padded virtual slots are zero so they only dilute by a known
    # count (divide by N, not NV).
    hsum = np.zeros(DIM, np.float64)
    sqsum = np.zeros(DIM, np.float64)
    for c in range(CORES):
        hp = np.asarray(results1[c]["hpre"]).astype(np.float64)
        hsum += hp.sum(axis=0)
        sqsum += (hp * hp).sum(axis=0)
    mean = hsum / N
    var = sqsum / N - mean * mean
    scale = (np.asarray(gamma, np.float64) / np.sqrt(var + EPS_BN))
    shift = np.asarray(beta, np.float64) - mean * scale

    scT = np.ascontiguousarray(
        scale.astype(np.float32).reshape(2, 128).T)
    shT = np.ascontiguousarray(
        shift.astype(np.float32).reshape(2, 128).T)

    nc2 = _get_phase2()
    in_maps2 = []
    for c in range(CORES):
        hp = np.asarray(results1[c]["hpre"])            # [NPC, 256] bf16
        hT = np.ascontiguousarray(
            hp.T.reshape(2, 128, NPC).transpose(1, 0, 2))
        in_maps2.append(dict(hT=hT, scaleT=scT, shiftT=shT))
    r2 = run_bass_kernel_spmd(nc2, in_maps2, list(range(CORES)), trace=trace)
    t2 = r2.exec_time_ns

    hv = np.concatenate([
        np.asarray(r2.results[c]["outT"]).transpose(1, 0, 2)
        .reshape(DIM, NPC).T
        for c in range(CORES)])                         # [NV, 256] fp16
    out = hv[prep["vid"]]
    info = dict(t1=t1, t2=t2, tpw=tpw)
    return np.ascontiguousarray(out.astype(np.float32)), info


def kernel(x, edge_attr, WQ, WK, WE, WV, gamma, beta, edge_index):
    out, _ = run_pipeline(x, edge_attr, WQ, WK, WE, WV, gamma, beta, edge_index)
    return out


# revision 23
# speedup vs baseline: 1.0269x; 1.0269x over previous
"""Trainium2 Bass kernel v3 for nn_AttentionLayer (sparse graph attention + BN).

Strategy (8 cores, SPMD):
  - Host assigns nodes to 784 windows of 128 slots (serpentine deal by degree,
    plus a repair pass) so every window owns <= tpw*128 incident edges; 98
    windows per core. Output rows are produced in this "virtual" order and
    un-permuted on the host at the end.
  - Blob is stored partition-major ([W, 128, tpw*896]) so each window's DMA is
    one fully-contiguous 1.1MB transfer (8960B per partition line).
  - Device pipeline per 2-tile group: 6 bf16 matmuls/tile (KV, Q, EH
    projections + segment-sum), one scalar PSUM->SBUF copy for the EH|Q bank,
    score pipeline fused over 2 tiles on vector/scalar/gpsimd.
  - BatchNorm: per-core partial sums accumulated on the PE; host reduces
    across cores between the two launches; a second kernel applies the affine
    transform in transposed layout (per-partition scale/bias on the scalar
    engine), writing fp16.
"""

import math
import numpy as np
import ml_dtypes

import concourse.bass as bass
import concourse.tile as tile
from concourse import mybir
from concourse.bass_utils import run_bass_kernel_spmd

F32 = mybir.dt.float32
F16 = mybir.dt.float16
BF16 = mybir.dt.bfloat16

CORES = 8
N = 100000
E = 500000
DIM = 256
H = 8
DH = DIM // H
WPC = 98                  # windows per core
W = CORES * WPC           # 784 windows of 128 slots
NV = W * 128              # 100352 virtual node slots
NPC = WPC * 128           # 12544 node slots per core
EPS_Z = 1e-6
EPS_BN = 1e-5


# ----------------------------------------------------------------- host prep

def _balance_windows(deg):
    """Assign nodes to W windows of <=128 slots, balancing edge (degree) sums.
    Serpentine deal over degree-sorted nodes, then a swap repair pass.
    Returns (win_of_node, pos_of_node, max_sum)."""
    order = np.argsort(-deg, kind="stable")
    win_of = np.empty(N, dtype=np.int64)
    pos_of = np.empty(N, dtype=np.int64)
    rows = math.ceil(N / W)
    for r in range(rows):
        chunk = order[r * W:(r + 1) * W]
        cols = np.arange(chunk.shape[0])
        if r % 2 == 1:
            cols = W - 1 - cols
        win_of[chunk] = cols
        pos_of[chunk] = r
    sums = np.bincount(win_of, weights=deg, minlength=W).astype(np.int64)
    counts = np.bincount(win_of, minlength=W)

    cap = 128 * max(1, math.ceil(sums.mean() / 128))
    if sums.max() > cap:
        members = [list(np.where(win_of == w)[0]) for w in range(W)]
        for _ in range(5000):
            hi = int(np.argmax(sums))
            if sums[hi] <= cap:
                break
            need = sums[hi] - cap
            cands = sorted(members[hi], key=lambda n: deg[n])
            cand = next((n for n in cands if deg[n] >= need), cands[-1])
            d = deg[cand]
            blocked = (counts >= 128) | (sums + d > cap)
            blocked[hi] = True
            if blocked.all():
                break
            lo = int(np.argmin(np.where(blocked, np.iinfo(np.int64).max, sums)))
            members[hi].remove(cand)
            members[lo].append(cand)
            win_of[cand] = lo
            sums[hi] -= d
            sums[lo] += d
            counts[hi] -= 1
            counts[lo] += 1
    pos_of = np.zeros(N, dtype=np.int64)
    order2 = np.argsort(win_of, kind="stable")
    start = 0
    for w, c in enumerate(np.bincount(win_of, minlength=W)):
        pos_of[order2[start:start + c]] = np.arange(c)
        start += c
    return win_of, pos_of, int(sums.max())


def _prep(x, edge_attr, edge_index):
    src = np.asarray(edge_index[0], dtype=np.int64)
    dst = np.asarray(edge_index[1], dtype=np.int64)
    x = np.asarray(x, dtype=np.float32)
    edge_attr = np.asarray(edge_attr, dtype=np.float32)

    deg = np.bincount(dst, minlength=N).astype(np.int64)
    win_of, pos_of, max_sum = _balance_windows(deg)
    tpw = max(1, math.ceil(max_sum / 128))
    T = WPC * tpw            # tiles per core
    vid = win_of * 128 + pos_of

    # edge -> (core, tile, lane)
    wd = win_of[dst]
    order_e = np.argsort(wd, kind="stable")
    wds = wd[order_e]
    cnt = np.bincount(wd, minlength=W)
    starts = np.zeros(W, dtype=np.int64)
    starts[1:] = np.cumsum(cnt)[:-1]
    pos_in_w = np.arange(E, dtype=np.int64) - starts[wds]
    tile_in_w = pos_in_w >> 7
    lane = pos_in_w & 127
    core_e = wds // WPC
    flat_t = core_e * T + (wds % WPC) * tpw + tile_in_w  # [E] in sorted order

    TT_all = CORES * T
    srcs = np.zeros((TT_all, 128), dtype=np.int64)
    slots = np.zeros((TT_all, 128), dtype=np.int64)
    valid = np.zeros((TT_all, 128), dtype=bool)
    eidx = np.zeros((TT_all, 128), dtype=np.int64)
    srcs[flat_t, lane] = src[order_e]
    slots[flat_t, lane] = pos_of[dst[order_e]]
    eidx[flat_t, lane] = order_e
    valid[flat_t, lane] = True

    xbf = x.astype(ml_dtypes.bfloat16)
    eabf = edge_attr.astype(ml_dtypes.bfloat16)
    xdst_nodes = np.zeros(NV, dtype=np.int64)  # virtual slot -> node (0 pad ok)
    node_mask = np.zeros(NV, dtype=bool)
    xdst_nodes[vid] = np.arange(N)
    node_mask[vid] = True

    per_core = []
    for c in range(CORES):
        sl = slice(c * T, (c + 1) * T)
        sc, vv, ei = srcs[sl], valid[sl], eidx[sl]
        st = slots[sl]
        blob = np.zeros((T, 128, 896), dtype=ml_dtypes.bfloat16)
        xs = xbf[sc]                       # [T,128,256]
        blob[:, :, 0:256] = np.concatenate(
            (xs[:, :, 0:128].transpose(0, 2, 1),
             xs[:, :, 128:256].transpose(0, 2, 1)), axis=2)
        wglob = (np.arange(T) // tpw) + c * WPC
        vslot = wglob[:, None] * 128 + st
        dsts_c = xdst_nodes[vslot]
        xd = xbf[dsts_c]
        blob[:, :, 256:512] = np.concatenate(
            (xd[:, :, 0:128].transpose(0, 2, 1),
             xd[:, :, 128:256].transpose(0, 2, 1)), axis=2)
        ea = eabf[ei]
        blob[:, :, 512:768] = np.concatenate(
            (ea[:, :, 0:128].transpose(0, 2, 1),
             ea[:, :, 128:256].transpose(0, 2, 1)), axis=2)
        selb = np.zeros((T, 128, 128), dtype=ml_dtypes.bfloat16)
        tI, lI = np.nonzero(vv)
        selb[tI, lI, st[tI, lI]] = 1.0
        blob[:, :, 768:896] = selb

        # partition-major layout: [WPC, 128, tpw*896], contiguous per partition
        blob_pm = np.ascontiguousarray(
            blob.reshape(WPC, tpw, 128, 896).transpose(0, 2, 1, 3)
        ).reshape(WPC, 128, tpw * 896)

        vsl = slice(c * NPC, (c + 1) * NPC)
        xwin = np.zeros((NPC, DIM), dtype=ml_dtypes.bfloat16)
        m = node_mask[vsl]
        xwin[m] = xbf[xdst_nodes[vsl][m]]
        per_core.append(dict(blob=blob_pm, xwin=xwin))
    return dict(per_core=per_core, tpw=tpw, vid=vid)


# ------------------------------------------------------------- phase1 kernel

def _groups(tpw):
    g, t = [], 0
    while t < tpw:
        n = min(2, tpw - t)
        g.append((t, n))
        t += n
    return g


def _build_phase1(tpw, split_waits=True):
    from contextlib import ExitStack
    nc = bass.Bass()
    blob_d = nc.declare_dram_parameter("blob", [WPC, 128, tpw * 896], BF16,
                                       isOutput=False)
    xwin_d = nc.declare_dram_parameter("xwin", [NPC, DIM], BF16, isOutput=False)
    wk_d = nc.declare_dram_parameter("wk", [128, 2, DIM], BF16, isOutput=False)
    wv_d = nc.declare_dram_parameter("wv", [128, 2, DIM], BF16, isOutput=False)
    wq_d = nc.declare_dram_parameter("wq", [128, 2, DIM], BF16, isOutput=False)
    we_d = nc.declare_dram_parameter("we", [128, 2, DIM], BF16, isOutput=False)
    hpre_d = nc.declare_dram_parameter("hpre", [NPC, DIM], BF16, isOutput=True)

    mul = mybir.AluOpType.mult
    add = mybir.AluOpType.add
    GRP = _groups(tpw)
    NGW = len(GRP)           # groups per window
    NG = WPC * NGW           # total groups

    with tile.TileContext(nc) as tc, ExitStack() as ctx:
        const = ctx.enter_context(tc.tile_pool(name="const", bufs=1))
        wk_sb = const.tile([128, 2, DIM], BF16, tag="wk")
        nc.sync.dma_start(wk_sb[:], wk_d[:])
        wv_sb = const.tile([128, 2, DIM], BF16, tag="wv")
        nc.sync.dma_start(wv_sb[:], wv_d[:])
        wq_sb = const.tile([128, 2, DIM], BF16, tag="wq")
        nc.sync.dma_start(wq_sb[:], wq_d[:])
        we_sb = const.tile([128, 2, DIM], BF16, tag="we")
        nc.sync.dma_start(we_sb[:], we_d[:])

        # PSUM: K (1 bank)x3 + V jit (1 bank)x2 + eq (2 banks)x1 + wvz = 8.
        # Measured Pareto-optimum: eq x2 (via pv=1) and wvz x2 (via pk=2)
        # both regress -- the jit V tile and the deep K buffering are worth
        # more than relieving the eq-copy cycle or the window boundary.
        pk = ctx.enter_context(tc.tile_pool(name="pk", bufs=3, space="PSUM"))
        pv = ctx.enter_context(tc.tile_pool(name="pv", bufs=2, space="PSUM"))
        peq = ctx.enter_context(tc.tile_pool(name="peq", bufs=1, space="PSUM"))
        pwvz = ctx.enter_context(tc.tile_pool(name="pwvz", bufs=1, space="PSUM"))

        p_blob = ctx.enter_context(tc.tile_pool(name="p_blob", bufs=3))
        p_eq = ctx.enter_context(tc.tile_pool(name="p_eq", bufs=4))
        p_m = ctx.enter_context(tc.tile_pool(name="p_m", bufs=5))
        p_small = ctx.enter_context(tc.tile_pool(name="p_small", bufs=6))
        p_msgz = ctx.enter_context(tc.tile_pool(name="p_msgz", bufs=4))
        p_h = ctx.enter_context(tc.tile_pool(name="p_h", bufs=3))
        p_xw = ctx.enter_context(tc.tile_pool(name="p_xw", bufs=3))

        def mm(out, lhsT, rhs, start, stop, **kw):
            nc.tensor.matmul(out, lhsT, rhs, start=start, stop=stop, **kw)

        state = {}

        def gidx(g):
            w, j = divmod(g, NGW)
            t0, n = GRP[j]
            return w, j, t0, n

        # stage A+B: window DMA + projections for the group's tiles
        def stage_B(g):
            w, j, t0, n = gidx(g)
            if j == 0:
                bw = p_blob.tile([128, tpw, 896], BF16, tag="blob")
                nc.sync.dma_start(
                    bw[:].rearrange("p t f -> p (t f)"), blob_d[w])
                xw = p_xw.tile([128, DIM], BF16, tag="xw")
                nc.sync.dma_start(xw[:], xwin_d[w * 128:(w + 1) * 128, :])
                state[("bw", w)] = bw
                state[("xw", w)] = xw
            bw = state[("bw", w)]
            k2 = pk.tile([128, 2, DIM], F32, tag="k2")
            eq2 = peq.tile([128, 2, 2 * DIM], F32, tag="eq2")
            for i in range(n):
                b = bw[:, t0 + i, :]
                mm(k2[:, i, :], b[:, 0:128], wk_sb[:, 0, :], True, False)
                mm(k2[:, i, :], b[:, 128:256], wk_sb[:, 1, :], False, True)
            for i in range(n):
                b = bw[:, t0 + i, :]
                mm(eq2[:, i, 0:DIM], b[:, 512:640], we_sb[:, 0, :], True, False)
                mm(eq2[:, i, 0:DIM], b[:, 640:768], we_sb[:, 1, :], False, True)
            for i in range(n):
                b = bw[:, t0 + i, :]
                mm(eq2[:, i, DIM:2 * DIM], b[:, 256:384], wq_sb[:, 0, :], True, False)
                mm(eq2[:, i, DIM:2 * DIM], b[:, 384:512], wq_sb[:, 1, :], False, True)
            state[("k2", g)] = k2
            state[("eq2", g)] = eq2

        # stage C: scalar copies EH|Q bank to SBUF
        def stage_C(g):
            w, j, t0, n = gidx(g)
            eq2 = state.pop(("eq2", g))
            eq_sb = p_eq.tile([128, 2, 2 * DIM], BF16, tag="eqsb")
            nc.scalar.copy(eq_sb[:, 0:n, :], eq2[:, 0:n, :])
            state[("eqsb", g)] = eq_sb

        # stage D: vector computes m1 = K * EH
        def stage_D(g):
            w, j, t0, n = gidx(g)
            k2 = state.pop(("k2", g))
            eq_sb = state[("eqsb", g)]
            m1 = p_m.tile([128, 2, DIM], BF16, tag="m1")
            nc.vector.tensor_tensor(out=m1[:, 0:n, :], in0=k2[:, 0:n, :],
                                    in1=eq_sb[:, 0:n, 0:DIM], op=mul)
            state[("m1", g)] = m1

        # stage E..H: gpsimd s2, vector reduce, gpsimd clip, scalar exp
        def stage_EH(g):
            w, j, t0, n = gidx(g)
            eq_sb = state.pop(("eqsb", g))
            m1 = state.pop(("m1", g))
            s2 = p_m.tile([128, 2, DIM], BF16, tag="s2")
            nc.gpsimd.tensor_tensor(out=s2[:, 0:n, :], in0=m1[:, 0:n, :],
                                    in1=eq_sb[:, 0:n, DIM:2 * DIM], op=mul)
            hs = p_small.tile([128, 2, H], F32, tag="hs")
            nc.vector.tensor_reduce(
                out=hs[:, 0:n, :, None],
                in_=s2[:, 0:n, :].rearrange("p n (h d) -> p n h d", d=DH),
                op=add, axis=mybir.AxisListType.X)
            hc = p_small.tile([128, 2, H], F32, tag="hc")
            nc.gpsimd.tensor_scalar(out=hc[:, 0:n, :], in0=hs[:, 0:n, :],
                                    scalar1=5.0, scalar2=-5.0,
                                    op0=mybir.AluOpType.min,
                                    op1=mybir.AluOpType.max)
            msgz = p_msgz.tile([128, 2, DIM + H], BF16, tag="msgz")
            nc.scalar.activation(msgz[:, 0:n, DIM:DIM + H], hc[:, 0:n, :],
                                 mybir.ActivationFunctionType.Exp)
            state[("msgz", g)] = msgz

        # stage I: PE (re)projects V just-in-time, vector applies scores
        def stage_I(g):
            w, j, t0, n = gidx(g)
            bw = state[("bw", w)]
            msgz = state.pop(("msgz", g))
            v2 = pv.tile([128, 2, DIM], F32, tag="v2")
            for i in range(n):
                b = bw[:, t0 + i, :]
                mm(v2[:, i, :], b[:, 0:128], wv_sb[:, 0, :], True, False)
                mm(v2[:, i, :], b[:, 128:256], wv_sb[:, 1, :], False, True)
            nc.vector.tensor_tensor(
                out=msgz[:, 0:n, 0:DIM].rearrange("p n (h d) -> p n h d", d=DH),
                in0=v2[:, 0:n, :].rearrange("p n (h d) -> p n h d", d=DH),
                in1=msgz[:, 0:n, DIM:DIM + H, None].to_broadcast([128, n, H, DH]),
                op=mul)
            state[("msgzf", g)] = msgz

        # stage J: segment-sum matmuls, one slot later so they never sit at
        # the PE queue head waiting for the msg-mult.
        def stage_J(g):
            w, j, t0, n = gidx(g)
            msgz = state.pop(("msgzf", g))
            if j == 0:
                state[("wvz", w)] = pwvz.tile([128, DIM + H], F32, tag="wvz",
                                              name="wvz")
            wvz = state[("wvz", w)]
            bw = state[("bw", w)]
            for i in range(n):
                t = t0 + i
                mm(wvz[:], bw[:, t, 768:896], msgz[:, i, :],
                   t == 0, t == tpw - 1)
            if j == NGW - 1:
                finalize(w)

        def finalize(w):
            wvz = state.pop(("wvz", w))
            state.pop(("bw", w))
            xw = state.pop(("xw", w))
            zr = p_small.tile([128, H], F32, tag="zr")
            nc.vector.tensor_scalar(out=zr[:], in0=wvz[:, DIM:DIM + H],
                                    scalar1=EPS_Z, scalar2=None, op0=add)
            nc.vector.reciprocal(zr[:], zr[:])
            h = p_h.tile([128, DIM], BF16, tag="h")
            nc.vector.tensor_tensor(
                out=h[:].rearrange("p (h d) -> p h d", d=DH),
                in0=wvz[:, 0:DIM].rearrange("p (h d) -> p h d", d=DH),
                in1=zr[:, :, None].to_broadcast([128, H, DH]), op=mul)
            nc.gpsimd.tensor_tensor(out=h[:], in0=h[:], in1=xw[:], op=add)
            nc.sync.dma_start(hpre_d[w * 128:(w + 1) * 128, :], h[:])

        # software pipeline over groups: B@0, C/D@1, EH/I@2, J@3. Issue order
        # per slot keeps each engine's FIFO free of head-of-line blocking:
        # the scalar copy C(i-1) is ready immediately and precedes exp; the
        # vector queue runs reduce(i-2), msgz(i-2) before m1(i-1) (which
        # waits on C); the PE runs V-mms(i-2), then the new projections, and
        # only then seg-sums(i-3), whose msg inputs are long since done.
        for i in range(NG + 3):
            if 1 <= i <= NG:
                stage_C(i - 1)
            if 2 <= i <= NG + 1:
                stage_EH(i - 2)
            if 2 <= i <= NG + 1:
                stage_I(i - 2)
            if 1 <= i <= NG:
                stage_D(i - 1)
            if i < NG:
                stage_B(i)
            if i >= 3:
                stage_J(i - 3)

    return _split_excess_waits(nc) if split_waits else nc


def _split_excess_waits(nc, max_waits=1):
    """Most HW-decoded opcodes carry only ~1 sync wait; move the excess onto
    preceding same-engine NoOps, which use the sequencer wait table."""
    k = 0
    skip = {"InstNoOp"}
    for f in nc.m.functions:
        for b in f.blocks:
            new = []
            for inst in b.instructions:
                si = inst.sync_info
                if (type(inst).__name__ not in skip and si is not None
                        and si.on_wait and len(si.on_wait) > max_waits):
                    extra = si.on_wait[:-max_waits]
                    for wt in extra:
                        nop = mybir.InstNoOp(name=f"I-wsplit{k}", ins=[], outs=[])
                        k += 1
                        nop.engine = inst.engine
                        nop.bass_nofuse = True
                        nop.sync_info = mybir.SyncInfo(on_wait=[wt], on_update=[])
                        new.append(nop)
                    inst.sync_info = mybir.SyncInfo(
                        on_wait=si.on_wait[-max_waits:], on_update=si.on_update)
                new.append(inst)
            b.instructions = new
    return nc


# ------------------------------------------------------------- phase2 kernel
# BN affine apply in transposed layout: hT[p, c, node] = h[node, c*128+p].
# scale/shift become per-partition vectors -> one scalar-engine ACTIVATE per
# (chunk c, node-slice), out = Identity(scale*in + bias) written as fp16.

P2_CHUNKS = 8


def _build_phase2():
    from contextlib import ExitStack
    nc = bass.Bass()
    hT_d = nc.declare_dram_parameter("hT", [128, 2, NPC], BF16, isOutput=False)
    sc_d = nc.declare_dram_parameter("scaleT", [128, 2], F32, isOutput=False)
    sh_d = nc.declare_dram_parameter("shiftT", [128, 2], F32, isOutput=False)
    out_d = nc.declare_dram_parameter("outT", [128, 2, NPC], F16, isOutput=True)
    CH = NPC // P2_CHUNKS
    assert CH * P2_CHUNKS == NPC
    with tile.TileContext(nc) as tc, ExitStack() as ctx:
        const = ctx.enter_context(tc.tile_pool(name="const", bufs=1))
        sc = const.tile([128, 2], F32, tag="sc")
        nc.sync.dma_start(sc[:], sc_d[:])
        sh = const.tile([128, 2], F32, tag="sh")
        nc.sync.dma_start(sh[:], sh_d[:])
        pool = ctx.enter_context(tc.tile_pool(name="ht", bufs=4))
        for b in range(P2_CHUNKS):
            ht = pool.tile([128, 2, CH], BF16, tag="ht")
            nc.sync.dma_start(ht[:], hT_d[:, :, b * CH:(b + 1) * CH])
            hf = pool.tile([128, 2, CH], F16, tag="hf")
            for c in range(2):
                nc.scalar.activation(hf[:, c, :], ht[:, c, :],
                                     mybir.ActivationFunctionType.Identity,
                                     bias=sh[:, c:c + 1], scale=sc[:, c:c + 1])
            nc.gpsimd.dma_start(out_d[:, :, b * CH:(b + 1) * CH], hf[:])
    return _split_excess_waits(nc)


# ------------------------------------------------------------------- runner

def _install_ntff_hook():
    import sys, types
    if "antenv.axon_hooks" in sys.modules:
        return True
    try:
        import antenv
        from trn_agent_boot.trn_boot import _ntff_profile_via_ctypes
        mod = types.ModuleType("antenv.axon_hooks")
        mod._hook = _ntff_profile_via_ctypes("/opt/axon/libaxon_pjrt.so")
        mod.set_axon_ntff_profile_hook = lambda h: setattr(mod, "_hook", h)
        mod.get_axon_ntff_profile_hook = lambda: mod._hook
        sys.modules["antenv.axon_hooks"] = mod
        antenv.axon_hooks = mod
        return mod._hook is not None
    except Exception:
        return False


_CACHE = {}


def _get_phase1(tpw):
    key = ("p1", tpw)
    if key not in _CACHE:
        _CACHE[key] = _build_phase1(tpw)
    return _CACHE[key]


def _get_phase2():
    key = ("p2",)
    if key not in _CACHE:
        _CACHE[key] = _build_phase2()
    return _CACHE[key]


def run_pipeline(x, edge_attr, WQ, WK, WE, WV, gamma, beta, edge_index,
                 timed=False):
    prep = _prep(x, edge_attr, edge_index)
    tpw = prep["tpw"]
    scale_inv = np.float32(1.0 / math.sqrt(DH))

    def cast_pm(a, f):
        # [DIM, f] weights -> [128, 2, f] partition-major bf16
        return np.ascontiguousarray(
            np.asarray(a, np.float32).reshape(2, 128, f).transpose(1, 0, 2)
        ).astype(ml_dtypes.bfloat16)

    wq = cast_pm(np.asarray(WQ, np.float32) * scale_inv, DIM)
    we = cast_pm(WE, DIM)
    wk = cast_pm(WK, DIM)
    wv = cast_pm(WV, DIM)

    nc1 = _get_phase1(tpw)
    in_maps = []
    for c in range(CORES):
        pc = prep["per_core"][c]
        in_maps.append(dict(blob=pc["blob"], xwin=pc["xwin"],
                            wk=wk, wv=wv, wq=wq, we=we))

    trace = timed and _install_ntff_hook()
    r1 = run_bass_kernel_spmd(nc1, in_maps, list(range(CORES)), trace=trace)
    results1 = r1.results
    t1 = r1.exec_time_ns
    # BN batch stats: reduce the (host-visible) hpre partial sums across
    # cores; padded virtual slots are zero so they only dilute by a known
    # count (divide by N, not NV).
    hsum = np.zeros(DIM, np.float64)
    sqsum = np.zeros(DIM, np.float64)
    for c in range(CORES):
        hp = np.asarray(results1[c]["hpre"]).astype(np.float64)
        hsum += hp.sum(axis=0)
        sqsum += (hp * hp).sum(axis=0)
    mean = hsum / N
    var = sqsum / N - mean * mean
    scale = (np.asarray(gamma, np.float64) / np.sqrt(var + EPS_BN))
    shift = np.asarray(beta, np.float64) - mean * scale

    scT = np.ascontiguousarray(
        scale.astype(np.float32).reshape(2, 128).T)
    shT = np.ascontiguousarray(
        shift.astype(np.float32).reshape(2, 128).T)

    nc2 = _get_phase2()
    in_maps2 = []
    for c in range(CORES):
        hp = np.asarray(results1[c]["hpre"])            # [NPC, 256] bf16
        hT = np.ascontiguousarray(
            hp.T.reshape(2, 128, NPC).transpose(1, 0, 2))
        in_maps2.append(dict(hT=hT, scaleT=scT, shiftT=shT))
    r2 = run_bass_kernel_spmd(nc2, in_maps2, list(range(CORES)), trace=trace)
    t2 = r2.exec_time_ns

    hv = np.concatenate([
        np.asarray(r2.results[c]["outT"]).transpose(1, 0, 2)
        .reshape(DIM, NPC).T
        for c in range(CORES)])                         # [NV, 256] fp16
    out = hv[prep["vid"]]
    info = dict(t1=t1, t2=t2, tpw=tpw)
    return np.ascontiguousarray(out.astype(np.float32)), info


def kernel(x, edge_attr, WQ, WK, WE, WV, gamma, beta, edge_index):
    out, _ = run_pipeline(x, edge_attr, WQ, WK, WE, WV, gamma, beta, edge_index)
    return out
